# revision 2
# baseline (speedup 1.0000x reference)
"""MoE LoRA linear layer kernel for Trainium2, data-parallel over 8 NeuronCores.

Math (per token n):
    down = h @ down_w.T                      [N, 64]
    mask[n, r] = val[n, k] if idx[n, k] == r else 0   (indices distinct per row)
    out = (down * mask) @ up_w.T             [N, 4096]

Sharding: tokens split 8 ways (2048/core); LoRA weights replicated.

The kernel is HBM-bound (h in + out out dominate), so the design goal is
pure streaming at DMA line rate with all compute hidden underneath:

  * h is pre-transposed and pre-packed ON HOST into the exact SBUF image
    the down-projection wants ([i-chunk partitions, token free dim]) so
    every load is one fat contiguous 4 MB DMA and the PE never spends
    cycles transposing h (the old kernel burned ~half its PE time +
    most of DVE/ACT on 512 PE transposes and PSUM evacuations).
  * h and out travel as bf16 (host casts) -> DMA bytes halve: 16 MB in +
    16 MB out per core ~= 89 us at 358 GB/s/core HBM. PSUM accumulation
    stays fp32; measured rel err is well inside the 2e-2 gate.
  * the top-k scatter mask is materialized host-side (a layout transform
    of the idx/val tensors, [64, NT] bf16, 256 KB/core) and applied as a
    single elementwise multiply against the down-proj PSUM per tile.

Per-core pipeline (token tile TT=512 = 1 PSUM bank of free dim):
  1. load hT tile [128, 32*512] bf16 (one 4 MB contiguous DMA)
  2. 32 bf16 matmuls accumulate downT = dwT.T @ hT into PSUM [64, 512]
  3. resT [64, 512] bf16 = psum_dn * maskT (one DVE multiply per tile)
  4. up projection per 128-token chunk: 8x bf16 matmul [K=64, M=128,
     N=512] -> psum, copies alternate DVE/ACT into out_sb [128, 4096]
     bf16, single fat 1 MB store per chunk
"""

import sys

for p in ("/opt/trn_rl_repo", "/opt/pypackages"):
    if p not in sys.path:
        sys.path.insert(0, p)

import numpy as np
import ml_dtypes

BF16 = ml_dtypes.bfloat16

N, D_IN, D_OUT, RANK, TOPK = 16384, 4096, 4096, 64, 8
NCORES = 8
NT = N // NCORES          # tokens per core = 2048
P = 128                   # partitions
TT = 512                  # token tile (down-matmul free dim = 1 PSUM bank)
NKC = D_IN // P           # 32 contraction chunks for down proj
NJ = TT // P              # 4 x 128-token chunks per tile
NTILES = NT // TT         # 4 token tiles per core
OT = 512                  # output col tile (1 PSUM bank)
NOT = D_OUT // OT         # 8 output col tiles

_CACHE = {}


def _build_program():
    import concourse.bacc as bacc
    import concourse.mybir as mybir
    from concourse import tile

    f32 = mybir.dt.float32
    bf16 = mybir.dt.bfloat16
    # Bacc (not plain Bass): its finalize() runs move_matmul_waits_to_-
    # ldweights + generate_event_semaphores, which split semaphore waits to
    # satisfy the TRN2 one-wait-per-instruction constraint.
    nc = bacc.Bacc()

    ht = nc.declare_dram_parameter("ht", [NTILES * P, NKC * TT], bf16,
                                   isOutput=False)
    dwt = nc.declare_dram_parameter("dwt", [P, NKC * RANK], bf16,
                                    isOutput=False)
    upt = nc.declare_dram_parameter("upt", [RANK, D_OUT], bf16,
                                    isOutput=False)
    maskt = nc.declare_dram_parameter("maskt", [RANK, NT], bf16,
                                      isOutput=False)
    out = nc.declare_dram_parameter("out", [NT, D_OUT], bf16, isOutput=True)

    with tile.TileContext(nc) as tc:
        with (
            tc.tile_pool(name="const", bufs=1) as const,
            tc.tile_pool(name="hT", bufs=2) as hT_pool,
            tc.tile_pool(name="resT", bufs=2) as resT_pool,
            tc.tile_pool(name="outsb", bufs=3) as out_pool,
            tc.tile_pool(name="psum_dn", bufs=2, space="PSUM") as psum_dn_pool,
            tc.tile_pool(name="psum_up", bufs=6, space="PSUM") as psum_up_pool,
        ):
            dwt_sb = const.tile([P, NKC * RANK], bf16)
            upt_sb = const.tile([RANK, D_OUT], bf16)
            maskt_sb = const.tile([RANK, NT], bf16)
            nc.sync.dma_start(out=dwt_sb[:], in_=dwt[:, :])
            nc.sync.dma_start(out=upt_sb[:], in_=upt[:, :])
            nc.sync.dma_start(out=maskt_sb[:], in_=maskt[:, :])

            copy_engines = [nc.vector.tensor_copy, nc.scalar.copy]
            cp_i = 0

            for tt in range(NTILES):
                # 1. one fat contiguous load: hT tile [128, 32*512] bf16
                hT = hT_pool.tile([P, NKC * TT], bf16)
                nc.sync.dma_start(out=hT[:], in_=ht[tt * P:(tt + 1) * P, :])

                # 2. down projection, accumulated over NKC chunks (bf16)
                psum_dn = psum_dn_pool.tile([RANK, TT], f32)
                for ki in range(NKC):
                    nc.tensor.matmul(
                        psum_dn[:],
                        lhsT=dwt_sb[:, ki * RANK:(ki + 1) * RANK],
                        rhs=hT[:, ki * TT:(ki + 1) * TT],
                        start=(ki == 0),
                        stop=(ki == NKC - 1),
                    )

                # 3. apply the routed top-k mask in one multiply
                resT = resT_pool.tile([RANK, TT], bf16)
                nc.vector.tensor_mul(
                    resT[:],
                    psum_dn[:],
                    maskt_sb[:, tt * TT:(tt + 1) * TT],
                )

                # 4. up projection (bf16) + fat store per 128-token chunk
                for j in range(NJ):
                    jj = tt * NJ + j
                    out_sb = out_pool.tile([P, D_OUT], bf16)
                    for o in range(NOT):
                        psum_up = psum_up_pool.tile([P, OT], f32)
                        nc.tensor.matmul(
                            psum_up[:],
                            lhsT=resT[:, j * P:(j + 1) * P],
                            rhs=upt_sb[:, o * OT:(o + 1) * OT],
                            start=True,
                            stop=True,
                        )
                        cp = copy_engines[cp_i % 2]
                        cp_i += 1
                        cp(
                            out=out_sb[:, o * OT:(o + 1) * OT],
                            in_=psum_up[:],
                        )
                    nc.sync.dma_start(
                        out=out[jj * P:(jj + 1) * P, :],
                        in_=out_sb[:],
                    )

    # Run the Bacc pipeline (register alloc + wait splitting for the TRN2
    # one-wait-per-instruction constraint) before the module is serialized.
    nc.finalize()
    return nc


def _get_program():
    if "nc" not in _CACHE:
        _CACHE["nc"] = _build_program()
    return _CACHE["nc"]


def prepare_in_maps(hidden_states, down_w, up_w, top_k_values, top_k_indices):
    h = np.ascontiguousarray(hidden_states, dtype=np.float32)
    dw = np.ascontiguousarray(down_w, dtype=np.float32)
    uw = np.ascontiguousarray(up_w, dtype=np.float32)
    vals = np.ascontiguousarray(top_k_values, dtype=np.float32)
    idx = np.asarray(top_k_indices).astype(np.int64)

    # hT image: hT[c][tt*128 + p, ki*512 + n] = h[c*NT + tt*512 + n, ki*128 + p]
    ht = (
        h.astype(BF16)
        .reshape(NCORES, NTILES, TT, NKC, P)
        .transpose(0, 1, 4, 3, 2)
        .reshape(NCORES, NTILES * P, NKC * TT)
    )
    ht = np.ascontiguousarray(ht)

    # dwT[p, ki*64 + r] = dw[r, ki*128 + p]
    dwt = np.ascontiguousarray(
        dw.reshape(RANK, NKC, P).transpose(2, 1, 0).reshape(P, NKC * RANK)
    ).astype(BF16)
    upt = np.ascontiguousarray(uw.T).astype(BF16)  # [64, 4096]

    # dense routed mask [N, 64] -> per-core maskT [64, NT]
    mask = np.zeros((N, RANK), dtype=np.float32)
    mask[np.arange(N)[:, None], idx] = vals
    maskt_all = mask.astype(BF16)

    in_maps = []
    for c in range(NCORES):
        s = slice(c * NT, (c + 1) * NT)
        in_maps.append(
            {
                "ht": ht[c],
                "dwt": dwt,
                "upt": upt,
                "maskt": np.ascontiguousarray(maskt_all[s].T),
            }
        )
    return in_maps


def kernel(hidden_states, down_w, up_w, top_k_values, top_k_indices, **_kw):
    from concourse.bass_utils import run_bass_kernel_spmd

    nc = _get_program()
    in_maps = prepare_in_maps(
        hidden_states, down_w, up_w, top_k_values, top_k_indices
    )
    res = run_bass_kernel_spmd(nc, in_maps, core_ids=list(range(NCORES)))
    return np.concatenate(
        [r["out"].astype(np.float32) for r in res.results], axis=0
    )


# revision 9
# speedup vs baseline: 1.1022x; 1.1022x over previous
"""MoE LoRA linear layer kernel for Trainium2, data-parallel over 8 NeuronCores.

Math (per token n):
    down = h @ down_w.T                      [N, 64]
    mask[n, r] = val[n, k] if idx[n, k] == r else 0   (indices distinct per row)
    out = (down * mask) @ up_w.T             [N, 4096]

Sharding: tokens split 8 ways (2048/core); LoRA weights replicated.

The kernel is HBM-bound (h in + out out dominate), so the design goal is
pure streaming at DMA line rate with all compute hidden underneath:

  * h is pre-transposed and pre-packed ON HOST into the exact SBUF image
    the down-projection wants ([i-chunk partitions, token free dim]) so
    every load is one fat contiguous 4 MB DMA and the PE never spends
    cycles transposing h (the old kernel burned ~half its PE time +
    most of DVE/ACT on 512 PE transposes and PSUM evacuations).
  * h and out travel as bf16 (host casts) -> DMA bytes halve: 16 MB in +
    16 MB out per core ~= 89 us at 358 GB/s/core HBM. PSUM accumulation
    stays fp32; measured rel err is well inside the 2e-2 gate.
  * the top-k scatter mask is materialized host-side (a layout transform
    of the idx/val tensors, [64, NT] bf16, 256 KB/core) and applied as a
    single elementwise multiply against the down-proj PSUM per tile.

Per-core pipeline (token tile TT=512 = 1 PSUM bank of free dim):
  1. load hT tile [128, 32*512] bf16 as four 1 MB contiguous DMAs
     (finer grain -> first down-matmul starts ~4 us in, and a lagging
     load stalls the PE for less than a HAM re-throttle window)
  2. 32 bf16 matmuls accumulate downT = dwT.T @ hT into PSUM [64, 512]
  3. resT [64, 512] bf16 = psum_dn * maskT (one DVE multiply per tile)
  4. up projection per 128-token chunk: 8x bf16 matmul [K=64, M=128,
     N=512] -> 2-bank PSUM pairs [128, 1024], one fat cast-copy per
     pair alternating DVE/ACT into out_sb [128, 4096] bf16, single
     fat 1 MB store per chunk

Stores are triggered from the (otherwise idle) GpSimd SWDGE so their
semaphore waits never block the next tile's load triggers on the Sync
queue; loads keep the fast Sync HWDGE ring.
"""

import sys

for p in ("/opt/trn_rl_repo", "/opt/pypackages"):
    if p not in sys.path:
        sys.path.insert(0, p)

import numpy as np
import ml_dtypes

BF16 = ml_dtypes.bfloat16

N, D_IN, D_OUT, RANK, TOPK = 16384, 4096, 4096, 64, 8
NCORES = 8
NT = N // NCORES          # tokens per core = 2048
P = 128                   # partitions
TT = 512                  # token tile (down-matmul free dim = 1 PSUM bank)
NKC = D_IN // P           # 32 contraction chunks for down proj
NJ = TT // P              # 4 x 128-token chunks per tile
NTILES = NT // TT         # 4 token tiles per core
OT = 512                  # output col tile (1 PSUM bank)
NOT = D_OUT // OT         # 8 output col tiles
NQ = 4                    # load quarters per hT tile (1 MB each)
QK = NKC // NQ            # 8 contraction chunks per quarter

_CACHE = {}


def _build_program():
    import concourse.bacc as bacc
    import concourse.mybir as mybir
    from concourse import tile

    f32 = mybir.dt.float32
    bf16 = mybir.dt.bfloat16
    # Bacc (not plain Bass): its finalize() runs move_matmul_waits_to_-
    # ldweights + generate_event_semaphores, which split semaphore waits to
    # satisfy the TRN2 one-wait-per-instruction constraint.
    nc = bacc.Bacc()

    ht = nc.declare_dram_parameter("ht", [NTILES * NQ * P, QK * TT], bf16,
                                   isOutput=False)
    dwt = nc.declare_dram_parameter("dwt", [P, NKC * RANK], bf16,
                                    isOutput=False)
    upt = nc.declare_dram_parameter("upt", [RANK, D_OUT], bf16,
                                    isOutput=False)
    maskt = nc.declare_dram_parameter("maskt", [RANK, NT], bf16,
                                      isOutput=False)
    out = nc.declare_dram_parameter("out", [NT, D_OUT], bf16, isOutput=True)

    with tile.TileContext(nc) as tc:
        with (
            tc.tile_pool(name="const", bufs=1) as const,
            tc.tile_pool(name="hT", bufs=2) as hT_pool,
            tc.tile_pool(name="resT", bufs=2) as resT_pool,
            tc.tile_pool(name="outsb", bufs=3) as out_pool,
            tc.tile_pool(name="psum_dn", bufs=2, space="PSUM") as psum_dn_pool,
            tc.tile_pool(name="psum_up", bufs=3, space="PSUM") as psum_up_pool,
        ):
            dwt_sb = const.tile([P, NKC * RANK], bf16)
            upt_sb = const.tile([RANK, D_OUT], bf16)
            maskt_sb = const.tile([RANK, NT], bf16)
            nc.sync.dma_start(out=dwt_sb[:], in_=dwt[:, :])

            copy_engines = [nc.vector.tensor_copy, nc.scalar.copy]
            cp_i = 0

            for tt in range(NTILES):
                # 1. load hT tile [128, 32*512] bf16 in 1 MB quarters
                hT = hT_pool.tile([P, NKC * TT], bf16)
                for q in range(NQ):
                    row = (tt * NQ + q) * P
                    nc.sync.dma_start(
                        out=hT[:, q * QK * TT:(q + 1) * QK * TT],
                        in_=ht[row:row + P, :],
                    )
                if tt == 0:
                    # up-proj consts aren't needed for ~8 us; load them
                    # after tile 0's h quarters so the first down matmul
                    # starts as early as possible.
                    nc.sync.dma_start(out=upt_sb[:], in_=upt[:, :])
                    nc.sync.dma_start(out=maskt_sb[:], in_=maskt[:, :])

                # 2. down projection, accumulated over NKC chunks (bf16)
                psum_dn = psum_dn_pool.tile([RANK, TT], f32)
                for ki in range(NKC):
                    nc.tensor.matmul(
                        psum_dn[:],
                        lhsT=dwt_sb[:, ki * RANK:(ki + 1) * RANK],
                        rhs=hT[:, ki * TT:(ki + 1) * TT],
                        start=(ki == 0),
                        stop=(ki == NKC - 1),
                    )

                # 3. apply the routed top-k mask in one multiply
                resT = resT_pool.tile([RANK, TT], bf16)
                nc.vector.tensor_mul(
                    resT[:],
                    psum_dn[:],
                    maskt_sb[:, tt * TT:(tt + 1) * TT],
                )

                # 4. up projection (bf16) + fat store per 128-token chunk.
                #    2 matmuls share a 2-bank PSUM pair -> one fat copy,
                #    halving the DVE/ACT per-instruction overhead.
                for j in range(NJ):
                    jj = tt * NJ + j
                    out_sb = out_pool.tile([P, D_OUT], bf16)
                    for op in range(NOT // 2):
                        psum_up = psum_up_pool.tile([P, 2 * OT], f32)
                        for h2 in range(2):
                            o = op * 2 + h2
                            nc.tensor.matmul(
                                psum_up[:, h2 * OT:(h2 + 1) * OT],
                                lhsT=resT[:, j * P:(j + 1) * P],
                                rhs=upt_sb[:, o * OT:(o + 1) * OT],
                                start=True,
                                stop=True,
                            )
                        cp = copy_engines[cp_i % 2]
                        cp_i += 1
                        cp(
                            out=out_sb[:, op * 2 * OT:(op + 1) * 2 * OT],
                            in_=psum_up[:],
                        )
                    nc.gpsimd.dma_start(
                        out=out[jj * P:(jj + 1) * P, :],
                        in_=out_sb[:],
                    )

    # Run the Bacc pipeline (register alloc + wait splitting for the TRN2
    # one-wait-per-instruction constraint) before the module is serialized.
    nc.finalize()
    return nc


def _get_program():
    if "nc" not in _CACHE:
        _CACHE["nc"] = _build_program()
    return _CACHE["nc"]


def prepare_in_maps(hidden_states, down_w, up_w, top_k_values, top_k_indices):
    h = np.ascontiguousarray(hidden_states, dtype=np.float32)
    dw = np.ascontiguousarray(down_w, dtype=np.float32)
    uw = np.ascontiguousarray(up_w, dtype=np.float32)
    vals = np.ascontiguousarray(top_k_values, dtype=np.float32)
    idx = np.asarray(top_k_indices).astype(np.int64)

    # hT image, quarter-major so each 1 MB load is contiguous:
    # ht[c][(tt*NQ + q)*128 + p, kl*512 + n] = h[c*NT + tt*512 + n,
    #                                            (q*QK + kl)*128 + p]
    ht = (
        h.astype(BF16)
        .reshape(NCORES, NTILES, TT, NQ, QK, P)
        .transpose(0, 1, 3, 5, 4, 2)
        .reshape(NCORES, NTILES * NQ * P, QK * TT)
    )
    ht = np.ascontiguousarray(ht)

    # dwT[p, ki*64 + r] = dw[r, ki*128 + p]
    dwt = np.ascontiguousarray(
        dw.reshape(RANK, NKC, P).transpose(2, 1, 0).reshape(P, NKC * RANK)
    ).astype(BF16)
    upt = np.ascontiguousarray(uw.T).astype(BF16)  # [64, 4096]

    # dense routed mask [N, 64] -> per-core maskT [64, NT]
    mask = np.zeros((N, RANK), dtype=np.float32)
    mask[np.arange(N)[:, None], idx] = vals
    maskt_all = mask.astype(BF16)

    in_maps = []
    for c in range(NCORES):
        s = slice(c * NT, (c + 1) * NT)
        in_maps.append(
            {
                "ht": ht[c],
                "dwt": dwt,
                "upt": upt,
                "maskt": np.ascontiguousarray(maskt_all[s].T),
            }
        )
    return in_maps


def kernel(hidden_states, down_w, up_w, top_k_values, top_k_indices, **_kw):
    from concourse.bass_utils import run_bass_kernel_spmd

    nc = _get_program()
    in_maps = prepare_in_maps(
        hidden_states, down_w, up_w, top_k_values, top_k_indices
    )
    res = run_bass_kernel_spmd(nc, in_maps, core_ids=list(range(NCORES)))
    return np.concatenate(
        [r["out"].astype(np.float32) for r in res.results], axis=0
    )
